# revision 2
# baseline (speedup 1.0000x reference)
"""Trainium2 Bass kernel for nn_CompositeHeadA (ragged multi-head readout).

Math restructuring (vs the naive reference):
  * depth rows are sorted, so the ragged dispatch is 5 contiguous segments
    per batch row; head h reads latent tokens x[in_off[h] : in_off[h]+nt[h]]
    and writes output rows [out_off[h] : out_off[h]+nt[h]*s[h]].  Only the
    first L = sum(nt) tokens of x are ever read.
  * ConvTranspose1d(E,E,k=s,stride=s) + Linear folds into a single GEMM:
      (x @ Wd[:,:,k] + bd) @ Wl + bl == x @ (Wd[:,:,k] @ Wl) + (bd @ Wl + bl)
    so each head becomes x_seg @ Wfold_h (+ bias row), Wfold_h: [256, 17*s].

Distribution: data-parallel over the batch dim N -> one NeuronCore per row
(8 rows, 8 cores).  Each core gets its row's x-prefix pre-transposed
(contraction dim on SBUF partitions) packed together with the folded
weights into one DRAM tensor per 128-row K-chunk, so every matmul depends
on a single DMA.
"""

import numpy as np

FACTORS = (1, 1, 1, 4, 8)
V = 17
E = 128 * 2  # embed dim; K-chunks of 128
WCOL = None  # computed from segment widths


def _segments_from_depth(depth_row: np.ndarray):
    counts = np.array([(depth_row == d).sum() for d in range(1, 6)], dtype=np.int64)
    f = np.array(FACTORS, dtype=np.int64)
    nt = counts // f
    in_off = np.cumsum(nt) - nt
    out_len = nt * f
    out_off = np.cumsum(out_len) - out_len
    return nt, in_off, out_len, out_off


def _fold_weights(W1, b1, W2, b2, W3, b3, Wd4, bd4, Wl4, bl4, Wd5, bd5, Wl5, bl5):
    # Wall: [E, 17*3 + 17*4 + 17*8] = [E, 255]; ball: [255]
    Wf4 = np.einsum("iok,ov->ikv", Wd4, Wl4).reshape(Wd4.shape[0], -1)
    Wf5 = np.einsum("iok,ov->ikv", Wd5, Wl5).reshape(Wd5.shape[0], -1)
    bf4 = (bd4 @ Wl4 + bl4[None, :]).repeat(Wd4.shape[2], axis=0) if False else None
    # bias fold: per k the bias is (bd @ Wl + bl) -> same for every k
    b4 = np.tile(bd4 @ Wl4 + bl4, Wd4.shape[2])
    b5 = np.tile(bd5 @ Wl5 + bl5, Wd5.shape[2])
    Wall = np.concatenate([W1, W2, W3, Wf4, Wf5], axis=1).astype(np.float32)
    ball = np.concatenate([b1, b2, b3, b4, b5]).astype(np.float32)
    return np.ascontiguousarray(Wall), np.ascontiguousarray(ball)


def _tile_list(nt, in_off, out_off):
    """[(h, lat0, m, out_row0, N_h, wcol_h), ...] per 128-token tile."""
    widths = [V, V, V, V * FACTORS[3], V * FACTORS[4]]
    wcol = np.cumsum([0] + widths)[:-1]
    tiles = []
    for h in range(5):
        s = FACTORS[h]
        n = int(nt[h])
        i = 0
        while i < n:
            m = min(128, n - i)
            tiles.append(
                (h, int(in_off[h]) + i, m, int(out_off[h]) + (i) * s,
                 widths[h], int(wcol[h]))
            )
            i += m
    return tiles, widths, wcol


def _build_program(L, C, T, nt, in_off, out_off, any_bias, n_cores):
    from concourse import bacc
    import concourse.mybir as mybir
    import concourse.tile as tile

    F32 = mybir.dt.float32
    nc = bacc.Bacc("TRN2", target_bir_lowering=False, debug=False,
                   num_devices=n_cores)
    xw_d = nc.dram_tensor("xw", [2, 128, C], F32, kind="ExternalInput")
    if any_bias:
        bias_d = nc.dram_tensor("biasrow", [1, 255], F32, kind="ExternalInput")
    out_d = nc.dram_tensor("out", [T, V], F32, kind="ExternalOutput")

    tiles, widths, wcol = _tile_list(nt, in_off, out_off)
    ntiles_h = [max(0, -(-int(nt[h]) // 128)) for h in range(5)]

    with tile.TileContext(nc) as tc:
        with (
            tc.tile_pool(name="sb", bufs=1) as sb,
            tc.tile_pool(name="ps", bufs=8, space="PSUM") as ps,
        ):
            xw0 = sb.tile([128, C], F32)
            xw1 = sb.tile([128, C], F32)
            nc.gpsimd.dma_start(xw0[:], xw_d[0])
            nc.gpsimd.dma_start(xw1[:], xw_d[1])
            xw_sb = [xw0, xw1]
            if any_bias:
                brow = sb.tile([1, 255], F32)
                nc.gpsimd.dma_start(brow[:], bias_d[:])
                ones = sb.tile([1, 128], F32)
                nc.gpsimd.memset(ones[:], 1.0)

            # per-segment output staging in SBUF
            stg = {}
            for h in range(5):
                if ntiles_h[h]:
                    stg[h] = sb.tile([128, ntiles_h[h] * widths[h]], F32,
                                     tag=f"stg{h}", name=f"stg{h}")

            tctr = {h: 0 for h in range(5)}
            for (h, lat0, m, out_r0, N_h, wc) in tiles:
                acc = ps.tile([128, V * FACTORS[4]], F32, tag="acc")
                for c in range(2):
                    nc.tensor.matmul(
                        acc[0:m, 0:N_h],
                        xw_sb[c][:, lat0:lat0 + m],
                        xw_sb[c][:, L + wc:L + wc + N_h],
                        start=(c == 0),
                        stop=(c == 1 and not any_bias),
                    )
                if any_bias:
                    nc.tensor.matmul(
                        acc[0:m, 0:N_h],
                        ones[0:1, 0:m],
                        brow[0:1, wc:wc + N_h],
                        start=False,
                        stop=True,
                    )
                ti = tctr[h]
                nc.vector.tensor_copy(
                    stg[h][0:m, ti * N_h:(ti + 1) * N_h], acc[0:m, 0:N_h]
                )
                tctr[h] += 1

            # per-segment output DMA (full-128 tiles in one shot, tail apart)
            for h in range(5):
                if not ntiles_h[h]:
                    continue
                s = FACTORS[h]
                N_h = widths[h]
                n = int(nt[h])
                k_full = n // 128
                m_t = n % 128
                r0 = int(out_off[h])
                if k_full:
                    rows = k_full * 128 * s
                    dst = out_d[r0:r0 + rows, :].rearrange(
                        "(i p s) v -> p i (s v)", i=k_full, p=128, s=s
                    )
                    nc.sync.dma_start(dst, stg[h][:, 0:k_full * N_h].rearrange(
                        "p (i w) -> p i w", i=k_full
                    ))
                if m_t:
                    r1 = r0 + k_full * 128 * s
                    dst = out_d[r1:r1 + m_t * s, :].rearrange(
                        "(p s) v -> p (s v)", p=m_t, s=s
                    )
                    nc.sync.dma_start(
                        dst, stg[h][0:m_t, k_full * N_h:(k_full + 1) * N_h]
                    )

            # zero-fill any tail beyond the dispatched region
            out_total = int(out_off[4] + nt[4] * FACTORS[4])
            if out_total < T:
                z = sb.tile([128, V], F32)
                nc.gpsimd.memset(z[:], 0.0)
                r = out_total
                while r < T:
                    m = min(128, T - r)
                    nc.sync.dma_start(out_d[r:r + m, :], z[0:m, :])
                    r += m

    nc.compile()
    return nc


def kernel(x, value, depth, position, W1, b1, W2, b2, W3, b3,
           Wd4, bd4, Wl4, bl4, Wd5, bd5, Wl5, bl5, **_unused):
    from concourse.bass_utils import run_bass_kernel_spmd

    x = np.asarray(x, dtype=np.float32)
    depth = np.asarray(depth)
    N, T, Edim = x.shape

    Wall, ball = _fold_weights(
        np.asarray(W1, np.float32), np.asarray(b1, np.float32),
        np.asarray(W2, np.float32), np.asarray(b2, np.float32),
        np.asarray(W3, np.float32), np.asarray(b3, np.float32),
        np.asarray(Wd4, np.float32), np.asarray(bd4, np.float32),
        np.asarray(Wl4, np.float32), np.asarray(bl4, np.float32),
        np.asarray(Wd5, np.float32), np.asarray(bd5, np.float32),
        np.asarray(Wl5, np.float32), np.asarray(bl5, np.float32),
    )
    any_bias = bool(np.any(ball != 0.0))

    # per-row segment schedules; group rows with identical schedules
    segs = [_segments_from_depth(np.asarray(depth[n])) for n in range(N)]
    sigs = [tuple(s[0].tolist()) for s in segs]
    groups = {}
    for n, sig in enumerate(sigs):
        groups.setdefault(sig, []).append(n)

    out_full = np.zeros((N, T, V), dtype=np.float32)

    for sig, rows in groups.items():
        nt, in_off, out_len, out_off = segs[rows[0]]
        L = int(nt.sum())
        C = L + 255
        # device program specialized on this schedule
        for batch_start in range(0, len(rows), 8):
            batch = rows[batch_start:batch_start + 8]
            nc = _build_program(L, C, T, nt, in_off, out_off, any_bias,
                                n_cores=len(batch))
            in_maps = []
            for n in batch:
                xT = np.ascontiguousarray(x[n, :L, :].T)  # [E, L]
                xw = np.concatenate(
                    [xT.reshape(2, 128, L), Wall.reshape(2, 128, 255)], axis=2
                ).astype(np.float32)
                im = {"xw": np.ascontiguousarray(xw)}
                if any_bias:
                    im["biasrow"] = ball.reshape(1, 255)
                in_maps.append(im)
            res = run_bass_kernel_spmd(nc, in_maps, list(range(len(batch))))
            for i, n in enumerate(batch):
                out_full[n] = res.results[i]["out"]

    return out_full
